# revision 10
# baseline (speedup 1.0000x reference)
"""Trainium2 Bass kernel for nn_BayesianKalmanNet_69621419868441.

Key structural insight: the reference's GRU "ensemble" (S=10) is degenerate.
All members start at h0=0 and receive identical inputs every step (the input
network is shared across samples, dropout is eval-mode), so every ensemble
member stays identical for the whole rollout. Hence:
  - x_filt == the single-member filtered state (mean of identical values),
  - P      == covariance of identical members == 0 (reference: |P| < 3e-13,
              pure fp rounding noise of the mean-subtraction).
The kernel computes the single-member recurrence and returns zeros for P.

Sharding: data-parallel over batch. B=256 -> 32 rows per core on 8 cores,
weights replicated, no collectives. Every per-step tensor lives
feature-on-partitions / batch-on-free ("transposed" layout), so the whole
recurrence runs without a single on-chip transpose. Engine base-partition
constraints (operands must start at partition 0) are met by stacking the
dx / innov groups along *columns* ([4, 64] tiles: cols 0:32 dx, 32:64 innov).

Per-core, per-step pipeline (T=64 serial steps):
  x_pred^T[4,32] = F^T-mm(xf);  y_pred^T[4,32] = (HF)^T-mm(xf) (rows 2:4 = 0)
  innov = y_t - y_pred  -> nnr[:,32:64]   (nnr[:,0:32] = dx from prev tail)
  sq = nnr*nnr; ss[1,64] = ones4-mm(sq)   (per-group sum of squares)
  rs[1,64] = rsqrt(max(ss,1e-24))  -- Quake bit-trick + 2 Newton steps on the
        DVE int/float ALU (an ACT rsqrt would force a ~2.7us activation-table
        reload per step since sigmoid/tanh live in a different table set)
  rsexp[4,64] = ones-mm(rs); nn = nnr*rsexp          (l2 normalize)
  a^T[256,32] = relu(Win-mms(nn));  relu = DVE max(x,0)
  gxh[768,32] in one PSUM bank = Whh-mms(h) + Wih-mms(a)  (r,z fused; hn sep)
  xn[256,32]  = Wih_n-mms(a)
  r,z = sigmoid(gxh_rz); n = tanh(xn + r*hn); h' = n + z*(h-n)
  K^T[4,64] = Wout-mms(h') (col-blocks per o); iexp = innov-expand-mms
  x_filt' = x_pred + K*.iexp summed over o;  dx' = x_filt' - x_pred
H is padded 200->256 (3H: 600->768) so all big matmuls use full 128-row
contraction chunks; padded rows are exactly zero and stay zero.
"""

import numpy as np
from contextlib import ExitStack

import concourse.bass as bass
import concourse.bacc as bacc
import concourse.tile as tile
from concourse import mybir

AF = mybir.ActivationFunctionType
OP = mybir.AluOpType
DT = mybir.dt

B, T, S, D, O, H = 256, 64, 10, 4, 2, 200
HP = 256                  # padded hidden
NCORES = 8
BL = B // NCORES          # 32 batch rows per core
MAGIC = 0x5F3759DF        # Quake rsqrt seed constant

_CACHE = {}


# --------------------------------------------------------------------------
# Host-side weight preprocessing (shared by all cores)
# --------------------------------------------------------------------------
def _prep_weights(F_mat, H_mat, W_in, b_in, W_ih, W_hh, b_ih, b_hh, W_out, b_out):
    F_mat = np.asarray(F_mat, np.float32)
    H_mat = np.asarray(H_mat, np.float32)
    W_in = np.asarray(W_in, np.float32)
    W_ih = np.asarray(W_ih, np.float32)
    W_hh = np.asarray(W_hh, np.float32)
    W_out = np.asarray(W_out, np.float32)
    b_in = np.asarray(b_in, np.float32)
    b_ih = np.asarray(b_ih, np.float32)
    b_hh = np.asarray(b_hh, np.float32)
    b_out = np.asarray(b_out, np.float32)

    def pad_gate_rows(w):
        out = np.zeros((3 * HP, HP), np.float32)
        for g in range(3):
            out[g * HP : g * HP + H, :H] = w[g * H : (g + 1) * H]
        return out

    def chunk_T(wp):  # [3HP, HP] -> lhsT chunks [128, 2*3HP]
        wt = wp.T
        return np.concatenate([wt[:128], wt[128:]], axis=1).copy()

    WhhT = chunk_T(pad_gate_rows(W_hh))
    WihT = chunk_T(pad_gate_rows(W_ih))

    # W_in fan-in groups -> two lhsT tensors [4, 256] (K=4; innov rows padded)
    WinT_dx = np.zeros((D, HP), np.float32)
    WinT_dx[:, :H] = W_in.T[0:D]
    WinT_in = np.zeros((D, HP), np.float32)
    WinT_in[0:O, :H] = W_in.T[D : D + O]

    # W_out [8=(d*2+o), 200]: per-o lhsT blocks.
    # cols layout kc-major then o: [kc0-o0(4), kc0-o1(4), kc1-o0(4), kc1-o1(4)]
    Wo = W_out.reshape(D, O, H)
    blocks = []
    for kc in range(2):
        for o in range(O):
            blk = np.zeros((128, D), np.float32)
            seg = Wo[:, o, :].T  # [200, 4] hidden-major
            lo, hi = kc * 128, min((kc + 1) * 128, H)
            if hi > lo:
                blk[: hi - lo] = seg[lo:hi]
            blocks.append(blk)
    WoutT = np.concatenate(blocks, axis=1)  # [128, 16]

    FT = F_mat.T.copy()                                  # [4,4]
    HFT = np.zeros((D, D), np.float32)
    HFT[:, 0:O] = (H_mat @ F_mat).T                      # [4,4], cols 2:4 zero

    ones41 = np.ones((D, 1), np.float32)                 # ss reduce
    ones14 = np.ones((1, D), np.float32)                 # rs partition-expand
    # innov o-expand: iexp[:, o-block] = all rows = innov[o]
    Eo0 = np.zeros((D, D), np.float32); Eo0[0, :] = 1.0
    Eo1 = np.zeros((D, D), np.float32); Eo1[1, :] = 1.0

    def padg(v):
        out = np.zeros((3 * HP,), np.float32)
        for g in range(3):
            out[g * HP : g * HP + H] = v[g * H : (g + 1) * H]
        return out

    b_rzh = padg(b_ih + b_hh)
    b_hh_n = padg(b_hh)[2 * HP :]
    b_ih_n = padg(b_ih)[2 * HP :]
    b_in_p = np.zeros((HP,), np.float32)
    b_in_p[:H] = b_in
    b_out_o = b_out.reshape(D, O).T.astype(np.float32)   # [O, D]

    has_bias = bool(
        np.any(b_in) or np.any(b_ih) or np.any(b_hh) or np.any(b_out)
    )
    biases = dict(
        b_rz=np.ascontiguousarray(b_rzh[: 2 * HP].reshape(4, 128).T),
        b_hh_n=np.ascontiguousarray(b_hh_n.reshape(2, 128).T),
        b_ih_n=np.ascontiguousarray(b_ih_n.reshape(2, 128).T),
        b_in=np.ascontiguousarray(b_in_p.reshape(2, 128).T),
        b_out=np.ascontiguousarray(b_out_o),             # [O, D] lhsT rows
    )
    return dict(
        WhhT=WhhT, WihT=WihT, WinT_dx=WinT_dx, WinT_in=WinT_in,
        WoutT=WoutT, FT=FT, HFT=HFT, ones41=ones41, ones14=ones14,
        Eo0=Eo0, Eo1=Eo1, has_bias=has_bias, biases=biases,
    )


# --------------------------------------------------------------------------
# Bass program
# --------------------------------------------------------------------------
def build_nc(has_bias: bool, num_devices: int = NCORES):
    nc = bacc.Bacc(
        "TRN2", target_bir_lowering=False, debug=False, num_devices=num_devices
    )
    f32 = DT.float32
    i32 = DT.int32

    din = {}
    def dram_in(name, shape):
        din[name] = nc.dram_tensor(name, list(shape), f32, kind="ExternalInput").ap()
        return din[name]

    y_in = dram_in("y_in", [D, T * BL])           # y^T padded to 4 rows
    WhhT_in = dram_in("WhhT", [128, 2 * 3 * HP])
    WihT_in = dram_in("WihT", [128, 2 * 3 * HP])
    WinTdx_in = dram_in("WinT_dx", [D, HP])
    WinTin_in = dram_in("WinT_in", [D, HP])
    WoutT_in = dram_in("WoutT", [128, 16])
    FT_in = dram_in("FT", [D, D])
    HFT_in = dram_in("HFT", [D, D])
    o41_in = dram_in("ones41", [D, 1])
    o14_in = dram_in("ones14", [1, D])
    Eo0_in = dram_in("Eo0", [D, D])
    Eo1_in = dram_in("Eo1", [D, D])
    if has_bias:
        brz_in = dram_in("b_rz", [128, 4])
        bhn_in = dram_in("b_hh_n", [128, 2])
        bxn_in = dram_in("b_ih_n", [128, 2])
        bin_in = dram_in("b_in", [128, 2])
        bout_in = dram_in("b_out", [O, D])

    xs_out = nc.dram_tensor("xs_out", [BL, T, D], f32, kind="ExternalOutput").ap()

    with tile.TileContext(nc) as tc, ExitStack() as ctx:
        cpool = ctx.enter_context(tc.tile_pool(name="const", bufs=1))
        spool = ctx.enter_context(tc.tile_pool(name="step", bufs=2))
        hpool = ctx.enter_context(tc.tile_pool(name="hstate", bufs=2))
        gpool = ctx.enter_context(tc.tile_pool(name="gates", bufs=2, space="PSUM"))
        mpool = ctx.enter_context(tc.tile_pool(name="mm", bufs=4, space="PSUM"))

        def load(name, ap, shape):
            t = cpool.tile(shape, f32, name=name)
            nc.sync.dma_start(t[:], ap[:])
            return t

        y_sb = load("y_sb", y_in, [D, T * BL])
        WhhT = load("WhhT_sb", WhhT_in, [128, 2 * 3 * HP])
        WihT = load("WihT_sb", WihT_in, [128, 2 * 3 * HP])
        WinT_dx = load("WinTdx_sb", WinTdx_in, [D, HP])
        WinT_in = load("WinTin_sb", WinTin_in, [D, HP])
        WoutT = load("WoutT_sb", WoutT_in, [128, 16])
        FT = load("FT_sb", FT_in, [D, D])
        HFT = load("HFT_sb", HFT_in, [D, D])
        ones41 = load("o41_sb", o41_in, [D, 1])
        ones14 = load("o14_sb", o14_in, [1, D])
        Eo0 = load("Eo0_sb", Eo0_in, [D, D])
        Eo1 = load("Eo1_sb", Eo1_in, [D, D])
        if has_bias:
            b_rz = load("brz_sb", brz_in, [128, 4])
            b_hh_n = load("bhn_sb", bhn_in, [128, 2])
            b_ih_n = load("bxn_sb", bxn_in, [128, 2])
            b_in_sb = load("bin_sb", bin_in, [128, 2])
            b_out_sb = load("bout_sb", bout_in, [O, D])
            ones32 = cpool.tile([1, BL], f32, name="ones32")
            nc.vector.memset(ones32[:], 1.0)

        # x_filt history: col 0:BL = x_filt_{-1} = 0
        xs_buf = cpool.tile([D, (T + 1) * BL], f32, name="xs_buf")
        nc.vector.memset(xs_buf[:, 0:BL], 0.0)

        h_cur = hpool.tile([128, 2 * BL], f32, name="h_state", tag="h")
        nc.vector.memset(h_cur[:], 0.0)

        # nnr: cols 0:32 dx (t=0: zero), cols 32:64 innov
        nnr = spool.tile([D, 2 * BL], f32, name="nnr", tag="nnr", bufs=3)
        nc.vector.memset(nnr[:, 0:BL], 0.0)

        for t in range(T):
            xf_prev = xs_buf[:, t * BL : (t + 1) * BL]

            # --- head ---
            xyp = mpool.tile([D, 2 * BL], f32, name=f"xyp{t}", tag="sm")
            nc.tensor.matmul(xyp[:, 0:BL], FT[:], xf_prev, start=True, stop=True)
            nc.tensor.matmul(xyp[:, BL : 2 * BL], HFT[:], xf_prev,
                             start=True, stop=True)
            xp_sb = spool.tile([D, BL], f32, name=f"xp{t}", tag="xp")
            nc.vector.tensor_copy(xp_sb[:], xyp[:, 0:BL])

            nc.vector.tensor_sub(
                nnr[:, BL : 2 * BL], y_sb[:, t * BL : (t + 1) * BL],
                xyp[:, BL : 2 * BL],
            )

            sq = spool.tile([D, 2 * BL], f32, name=f"sq{t}", tag="sq")
            nc.vector.tensor_mul(sq[:], nnr[:], nnr[:])
            ss = mpool.tile([1, 2 * BL], f32, name=f"ss{t}", tag="sm")
            nc.tensor.matmul(ss[:], ones41[:], sq[:], start=True, stop=True)

            # rs = rsqrt(max(ss, 1e-24)): bit-trick seed + 2 Newton steps
            ssc = spool.tile([1, 2 * BL], f32, name=f"ssc{t}", tag="ssc")
            nc.vector.tensor_single_scalar(ssc[:], ss[:], 1e-24, op=OP.max)
            yb = spool.tile([1, 2 * BL], f32, name=f"yb{t}", tag="yb")
            nc.vector.tensor_scalar(
                yb[:].bitcast(i32), ssc[:].bitcast(i32), 1, -1,
                op0=OP.logical_shift_right, op1=OP.bitwise_xor,
            )
            nc.vector.tensor_scalar_add(
                yb[:].bitcast(i32), yb[:].bitcast(i32), MAGIC + 1
            )
            u = spool.tile([1, 2 * BL], f32, name=f"u{t}", tag="u")
            nc.vector.tensor_scalar_mul(u[:], ssc[:], -0.5)
            v = spool.tile([1, 2 * BL], f32, name=f"v{t}", tag="v")
            for _ in range(2):
                nc.vector.tensor_mul(v[:], yb[:], yb[:])
                nc.vector.tensor_mul(v[:], v[:], u[:])
                nc.vector.tensor_scalar_add(v[:], v[:], 1.5)
                nc.vector.tensor_mul(yb[:], yb[:], v[:])

            rsexp = mpool.tile([D, 2 * BL], f32, name=f"rsexp{t}", tag="sm")
            nc.tensor.matmul(rsexp[:], ones14[:], yb[:], start=True, stop=True)
            nn = spool.tile([D, 2 * BL], f32, name=f"nn{t}", tag="nn")
            nc.vector.tensor_mul(nn[:], nnr[:], rsexp[:])

            # --- a = relu(W_in @ nn) ---
            aps = mpool.tile([128, 2 * BL], f32, name=f"aps{t}", tag="sm")
            nacc = 3 if has_bias else 2
            for kc in range(2):
                o = aps[:, kc * BL : (kc + 1) * BL]
                nc.tensor.matmul(o, WinT_dx[:, kc * 128 : (kc + 1) * 128],
                                 nn[:, 0:BL], start=True, stop=False)
                nc.tensor.matmul(o, WinT_in[:, kc * 128 : (kc + 1) * 128],
                                 nn[:, BL : 2 * BL], start=False,
                                 stop=(nacc == 2))
                if has_bias:
                    nc.tensor.matmul(o, b_in_sb[:, kc : kc + 1], ones32[:],
                                     start=False, stop=True)
            a_sb = spool.tile([128, 2 * BL], f32, name=f"a{t}", tag="a")
            nc.vector.tensor_single_scalar(a_sb[:], aps[:], 0.0, op=OP.max)

            # --- gates ---
            gxh = gpool.tile([128, 6 * BL], f32, name=f"gxh{t}", tag="g")
            nbias = 1 if has_bias else 0
            for mc in range(6):
                o = gxh[:, mc * BL : (mc + 1) * BL]
                n_acc = (4 if mc < 4 else 2) + nbias
                i = 0
                for kc in range(2):
                    nc.tensor.matmul(
                        o, WhhT[:, kc * 3 * HP + mc * 128 :][:, :128],
                        h_cur[:, kc * BL : (kc + 1) * BL],
                        start=(i == 0), stop=(i == n_acc - 1),
                    )
                    i += 1
                if mc < 4:
                    for kc in range(2):
                        nc.tensor.matmul(
                            o, WihT[:, kc * 3 * HP + mc * 128 :][:, :128],
                            a_sb[:, kc * BL : (kc + 1) * BL],
                            start=False, stop=(i == n_acc - 1),
                        )
                        i += 1
                if has_bias:
                    bsl = (b_rz[:, mc : mc + 1] if mc < 4
                           else b_hh_n[:, mc - 4 : mc - 3])
                    nc.tensor.matmul(o, bsl, ones32[:], start=False, stop=True)

            xn = mpool.tile([128, 2 * BL], f32, name=f"xn{t}", tag="sm")
            for mc in range(2):
                o = xn[:, mc * BL : (mc + 1) * BL]
                for kc in range(2):
                    nc.tensor.matmul(
                        o, WihT[:, kc * 3 * HP + (4 + mc) * 128 :][:, :128],
                        a_sb[:, kc * BL : (kc + 1) * BL],
                        start=(kc == 0), stop=(kc == 1 and not has_bias),
                    )
                if has_bias:
                    nc.tensor.matmul(o, b_ih_n[:, mc : mc + 1], ones32[:],
                                     start=False, stop=True)

            rz = spool.tile([128, 4 * BL], f32, name=f"rz{t}", tag="rz")
            nc.scalar.activation(rz[:], gxh[:, 0 : 4 * BL], AF.Sigmoid)

            npre = spool.tile([128, 2 * BL], f32, name=f"npre{t}", tag="npre")
            nc.vector.tensor_mul(npre[:], rz[:, 0 : 2 * BL],
                                 gxh[:, 4 * BL : 6 * BL])
            nc.vector.tensor_add(npre[:], npre[:], xn[:])
            n_sb = spool.tile([128, 2 * BL], f32, name=f"nsb{t}", tag="nsb")
            nc.scalar.activation(n_sb[:], npre[:], AF.Tanh)

            h_new = hpool.tile([128, 2 * BL], f32, name=f"h{t}", tag="h")
            d_sb = spool.tile([128, 2 * BL], f32, name=f"dsb{t}", tag="dsb")
            nc.vector.tensor_sub(d_sb[:], h_cur[:], n_sb[:])
            nc.vector.tensor_mul(d_sb[:], rz[:, 2 * BL : 4 * BL], d_sb[:])
            nc.vector.tensor_add(h_new[:], n_sb[:], d_sb[:])
            h_cur = h_new

            # --- K and x_filt ---
            KT = mpool.tile([D, 2 * BL], f32, name=f"KT{t}", tag="sm")
            for o in range(O):
                osl = KT[:, o * BL : (o + 1) * BL]
                for kc in range(2):
                    nc.tensor.matmul(
                        osl, WoutT[:, (kc * O + o) * D : (kc * O + o + 1) * D],
                        h_new[:, kc * BL : (kc + 1) * BL],
                        start=(kc == 0), stop=(kc == 1 and not has_bias),
                    )
                if has_bias:
                    nc.tensor.matmul(
                        osl, b_out_sb[o : o + 1, :], ones32[:],
                        start=False, stop=True,
                    )

            iexp = mpool.tile([D, 2 * BL], f32, name=f"iexp{t}", tag="sm")
            nc.tensor.matmul(iexp[:, 0:BL], Eo0[:], nnr[:, BL : 2 * BL],
                             start=True, stop=True)
            nc.tensor.matmul(iexp[:, BL : 2 * BL], Eo1[:], nnr[:, BL : 2 * BL],
                             start=True, stop=True)
            iexp_sb = spool.tile([D, 2 * BL], f32, name=f"iexpsb{t}", tag="ie")
            nc.vector.tensor_copy(iexp_sb[:], iexp[:])

            t8 = spool.tile([D, 2 * BL], f32, name=f"t8{t}", tag="t8")
            nc.vector.tensor_mul(t8[:], KT[:], iexp_sb[:])
            xe = spool.tile([D, BL], f32, name=f"xe{t}", tag="xe")
            nc.vector.tensor_add(xe[:], t8[:, 0:BL], t8[:, BL : 2 * BL])
            xf_new = xs_buf[:, (t + 1) * BL : (t + 2) * BL]
            nc.vector.tensor_add(xf_new, xe[:], xp_sb[:])

            nc.sync.dma_start(
                xs_out[:, t, :].rearrange("b d -> d b"), xf_new
            )

            if t + 1 < T:
                nnr = spool.tile([D, 2 * BL], f32, name=f"nnr{t + 1}",
                                 tag="nnr", bufs=3)
                nc.vector.tensor_sub(nnr[:, 0:BL], xf_new, xp_sb[:])

    nc.compile()
    return nc


# --------------------------------------------------------------------------
# Host entry
# --------------------------------------------------------------------------
def _make_in_maps(inputs):
    w = _prep_weights(
        inputs["F_mat"], inputs["H_mat"], inputs["W_in"], inputs["b_in"],
        inputs["W_ih"], inputs["W_hh"], inputs["b_ih"], inputs["b_hh"],
        inputs["W_out"], inputs["b_out"],
    )
    y_seq = np.asarray(inputs["y_seq"], np.float32)
    common = {
        "WhhT": w["WhhT"], "WihT": w["WihT"], "WinT_dx": w["WinT_dx"],
        "WinT_in": w["WinT_in"], "WoutT": w["WoutT"], "FT": w["FT"],
        "HFT": w["HFT"], "ones41": w["ones41"], "ones14": w["ones14"],
        "Eo0": w["Eo0"], "Eo1": w["Eo1"],
    }
    if w["has_bias"]:
        common.update(w["biases"])
    in_maps = []
    for c in range(NCORES):
        ys = y_seq[c * BL : (c + 1) * BL]              # [BL, T, O]
        yT = np.zeros((D, T * BL), np.float32)
        yT[0:O] = ys.transpose(2, 1, 0).reshape(O, T * BL)
        m = dict(common)
        m["y_in"] = yT
        in_maps.append(m)
    return in_maps, w["has_bias"]


def _get_nc(has_bias):
    key = ("nc", has_bias)
    if key not in _CACHE:
        _CACHE[key] = build_nc(has_bias)
    return _CACHE[key]


def kernel(**inputs):
    from concourse.bass_utils import run_bass_kernel_spmd

    in_maps, has_bias = _make_in_maps(inputs)
    nc = _get_nc(has_bias)
    res = run_bass_kernel_spmd(nc, in_maps, core_ids=list(range(NCORES)))
    xs = np.concatenate(
        [res.results[c]["xs_out"] for c in range(NCORES)], axis=0
    ).astype(np.float32)
    Ps = np.zeros((B, T, D, D), np.float32)
    return xs, Ps


# revision 13
# speedup vs baseline: 2.9318x; 2.9318x over previous
"""Trainium2 Bass kernel for nn_BayesianKalmanNet_69621419868441.

Key structural insight: the reference's GRU "ensemble" (S=10) is degenerate.
All members start at h0=0 and receive identical inputs every step (the input
network is shared across samples, dropout is eval-mode), so every ensemble
member stays identical for the whole rollout. Hence:
  - x_filt == the single-member filtered state (mean of identical values),
  - P      == covariance of identical members == 0 (reference: |P| < 3e-13,
              pure fp rounding noise of the mean-subtraction).
The kernel computes the single-member recurrence and returns zeros for P.

Sharding: data-parallel over batch. B=256 -> 32 rows per core on 8 cores,
weights replicated, no collectives. Every per-step tensor lives
feature-on-partitions / batch-on-free ("transposed" layout), so the whole
recurrence runs without a single on-chip transpose. Engine base-partition
constraints (operands must start at partition 0) are met by stacking the
dx / innov groups along *columns* ([4, 64] tiles: cols 0:32 dx, 32:64 innov).

Per-core, per-step pipeline (T=64 serial steps):
  x_pred^T[4,32] = F^T-mm(xf);  y_pred^T[4,32] = (HF)^T-mm(xf) (rows 2:4 = 0)
  innov = y_t - y_pred  -> nnr[:,32:64]   (nnr[:,0:32] = dx from prev tail)
  sq = nnr*nnr; ss[1,64] = ones4-mm(sq)   (per-group sum of squares)
  rs[1,64] = rsqrt(max(ss,1e-24))  -- Quake bit-trick + 2 Newton steps on the
        DVE int/float ALU (an ACT rsqrt would force a ~2.7us activation-table
        reload per step since sigmoid/tanh live in a different table set)
  rsexp[4,64] = ones-mm(rs); nn = nnr*rsexp          (l2 normalize)
  a^T[256,32] = relu(Win-mms(nn));  relu = DVE max(x,0)
  gxh[768,32] in one PSUM bank = Whh-mms(h) + Wih-mms(a)  (r,z fused; hn sep)
  xn[256,32]  = Wih_n-mms(a)
  r,z = sigmoid(gxh_rz); n = tanh(xn + r*hn); h' = n + z*(h-n)
  K^T[4,64] = Wout-mms(h') (col-blocks per o); iexp = innov-expand-mms
  x_filt' = x_pred + K*.iexp summed over o;  dx' = x_filt' - x_pred
H is padded 200->256 (3H: 600->768) so all big matmuls use full 128-row
contraction chunks; padded rows are exactly zero and stay zero.
"""

import numpy as np
from contextlib import ExitStack

import concourse.bass as bass
import concourse.bacc as bacc
import concourse.tile as tile
from concourse import mybir

AF = mybir.ActivationFunctionType
OP = mybir.AluOpType
DT = mybir.dt

B, T, S, D, O, H = 256, 64, 10, 4, 2, 200
HP = 256                  # padded hidden
NCORES = 8
BL = B // NCORES          # 32 batch rows per core
MAGIC = 0x5F3759DF        # Quake rsqrt seed constant

_CACHE = {}


# --------------------------------------------------------------------------
# Host-side weight preprocessing (shared by all cores)
# --------------------------------------------------------------------------
def _prep_weights(F_mat, H_mat, W_in, b_in, W_ih, W_hh, b_ih, b_hh, W_out, b_out):
    F_mat = np.asarray(F_mat, np.float32)
    H_mat = np.asarray(H_mat, np.float32)
    W_in = np.asarray(W_in, np.float32)
    W_ih = np.asarray(W_ih, np.float32)
    W_hh = np.asarray(W_hh, np.float32)
    W_out = np.asarray(W_out, np.float32)
    b_in = np.asarray(b_in, np.float32)
    b_ih = np.asarray(b_ih, np.float32)
    b_hh = np.asarray(b_hh, np.float32)
    b_out = np.asarray(b_out, np.float32)

    def pad_gate_rows(w):
        out = np.zeros((3 * HP, HP), np.float32)
        for g in range(3):
            out[g * HP : g * HP + H, :H] = w[g * H : (g + 1) * H]
        return out

    def chunk_T(wp):  # [3HP, HP] -> lhsT chunks [128, 2*3HP]
        wt = wp.T
        return np.concatenate([wt[:128], wt[128:]], axis=1).copy()

    WhhT = chunk_T(pad_gate_rows(W_hh))
    WihT = chunk_T(pad_gate_rows(W_ih))

    # W_in fan-in groups -> two lhsT tensors [4, 256] (K=4; innov rows padded)
    WinT_dx = np.zeros((D, HP), np.float32)
    WinT_dx[:, :H] = W_in.T[0:D]
    WinT_in = np.zeros((D, HP), np.float32)
    WinT_in[0:O, :H] = W_in.T[D : D + O]

    # W_out [8=(d*2+o), 200]: per-o lhsT blocks.
    # cols layout kc-major then o: [kc0-o0(4), kc0-o1(4), kc1-o0(4), kc1-o1(4)]
    Wo = W_out.reshape(D, O, H)
    blocks = []
    for kc in range(2):
        for o in range(O):
            blk = np.zeros((128, D), np.float32)
            seg = Wo[:, o, :].T  # [200, 4] hidden-major
            lo, hi = kc * 128, min((kc + 1) * 128, H)
            if hi > lo:
                blk[: hi - lo] = seg[lo:hi]
            blocks.append(blk)
    WoutT = np.concatenate(blocks, axis=1)  # [128, 16]

    FT = F_mat.T.copy()                                  # [4,4]
    HFT = np.zeros((D, D), np.float32)
    HFT[:, 0:O] = (H_mat @ F_mat).T                      # [4,4], cols 2:4 zero

    ones41 = np.ones((D, 1), np.float32)                 # ss reduce
    ones14 = np.ones((1, D), np.float32)                 # rs partition-expand
    # innov o-expand: iexp[:, o-block] = all rows = innov[o]
    Eo0 = np.zeros((D, D), np.float32); Eo0[0, :] = 1.0
    Eo1 = np.zeros((D, D), np.float32); Eo1[1, :] = 1.0

    def padg(v):
        out = np.zeros((3 * HP,), np.float32)
        for g in range(3):
            out[g * HP : g * HP + H] = v[g * H : (g + 1) * H]
        return out

    b_rzh = padg(b_ih + b_hh)
    b_hh_n = padg(b_hh)[2 * HP :]
    b_ih_n = padg(b_ih)[2 * HP :]
    b_in_p = np.zeros((HP,), np.float32)
    b_in_p[:H] = b_in
    b_out_o = b_out.reshape(D, O).T.astype(np.float32)   # [O, D]

    has_bias = bool(
        np.any(b_in) or np.any(b_ih) or np.any(b_hh) or np.any(b_out)
    )
    biases = dict(
        b_rz=np.ascontiguousarray(b_rzh[: 2 * HP].reshape(4, 128).T),
        b_hh_n=np.ascontiguousarray(b_hh_n.reshape(2, 128).T),
        b_ih_n=np.ascontiguousarray(b_ih_n.reshape(2, 128).T),
        b_in=np.ascontiguousarray(b_in_p.reshape(2, 128).T),
        b_out=np.ascontiguousarray(b_out_o),             # [O, D] lhsT rows
    )
    return dict(
        WhhT=WhhT, WihT=WihT, WinT_dx=WinT_dx, WinT_in=WinT_in,
        WoutT=WoutT, FT=FT, HFT=HFT, ones41=ones41, ones14=ones14,
        Eo0=Eo0, Eo1=Eo1, has_bias=has_bias, biases=biases,
    )


# --------------------------------------------------------------------------
# Bass program
# --------------------------------------------------------------------------
def build_nc(has_bias: bool, num_devices: int = NCORES):
    nc = bacc.Bacc(
        "TRN2", target_bir_lowering=False, debug=False, num_devices=num_devices
    )
    f32 = DT.float32
    i32 = DT.int32

    din = {}
    def dram_in(name, shape):
        din[name] = nc.dram_tensor(name, list(shape), f32, kind="ExternalInput").ap()
        return din[name]

    y_in = dram_in("y_in", [D, T * BL])           # y^T padded to 4 rows
    WhhT_in = dram_in("WhhT", [128, 2 * 3 * HP])
    WihT_in = dram_in("WihT", [128, 2 * 3 * HP])
    WinTdx_in = dram_in("WinT_dx", [D, HP])
    WinTin_in = dram_in("WinT_in", [D, HP])
    WoutT_in = dram_in("WoutT", [128, 16])
    FT_in = dram_in("FT", [D, D])
    HFT_in = dram_in("HFT", [D, D])
    o41_in = dram_in("ones41", [D, 1])
    o14_in = dram_in("ones14", [1, D])
    Eo0_in = dram_in("Eo0", [D, D])
    Eo1_in = dram_in("Eo1", [D, D])
    if has_bias:
        brz_in = dram_in("b_rz", [128, 4])
        bhn_in = dram_in("b_hh_n", [128, 2])
        bxn_in = dram_in("b_ih_n", [128, 2])
        bin_in = dram_in("b_in", [128, 2])
        bout_in = dram_in("b_out", [O, D])

    xs_out = nc.dram_tensor("xs_out", [BL, T, D], f32, kind="ExternalOutput").ap()

    with tile.TileContext(nc) as tc, ExitStack() as ctx:
        cpool = ctx.enter_context(tc.tile_pool(name="const", bufs=1))
        spool = ctx.enter_context(tc.tile_pool(name="step", bufs=2))
        hpool = ctx.enter_context(tc.tile_pool(name="hstate", bufs=2))
        gpool = ctx.enter_context(tc.tile_pool(name="gates", bufs=2, space="PSUM"))
        mpool = ctx.enter_context(tc.tile_pool(name="mm", bufs=4, space="PSUM"))

        def load(name, ap, shape):
            t = cpool.tile(shape, f32, name=name)
            nc.sync.dma_start(t[:], ap[:])
            return t

        y_sb = load("y_sb", y_in, [D, T * BL])
        WhhT = load("WhhT_sb", WhhT_in, [128, 2 * 3 * HP])
        WihT = load("WihT_sb", WihT_in, [128, 2 * 3 * HP])
        WinT_dx = load("WinTdx_sb", WinTdx_in, [D, HP])
        WinT_in = load("WinTin_sb", WinTin_in, [D, HP])
        WoutT = load("WoutT_sb", WoutT_in, [128, 16])
        FT = load("FT_sb", FT_in, [D, D])
        HFT = load("HFT_sb", HFT_in, [D, D])
        ones41 = load("o41_sb", o41_in, [D, 1])
        ones14 = load("o14_sb", o14_in, [1, D])
        Eo0 = load("Eo0_sb", Eo0_in, [D, D])
        Eo1 = load("Eo1_sb", Eo1_in, [D, D])
        if has_bias:
            b_rz = load("brz_sb", brz_in, [128, 4])
            b_hh_n = load("bhn_sb", bhn_in, [128, 2])
            b_ih_n = load("bxn_sb", bxn_in, [128, 2])
            b_in_sb = load("bin_sb", bin_in, [128, 2])
            b_out_sb = load("bout_sb", bout_in, [O, D])
            ones32 = cpool.tile([1, BL], f32, name="ones32")
            nc.vector.memset(ones32[:], 1.0)

        # x_filt history: col 0:BL = x_filt_{-1} = 0
        xs_buf = cpool.tile([D, (T + 1) * BL], f32, name="xs_buf")
        nc.vector.memset(xs_buf[:, 0:BL], 0.0)

        h_cur = hpool.tile([128, 2 * BL], f32, name="h_state", tag="h")
        nc.vector.memset(h_cur[:], 0.0)

        # nnr: cols 0:32 dx (t=0: zero), cols 32:64 innov
        nnr = spool.tile([D, 2 * BL], f32, name="nnr", tag="nnr", bufs=3)
        nc.vector.memset(nnr[:, 0:BL], 0.0)

        for t in range(T):
            xf_prev = xs_buf[:, t * BL : (t + 1) * BL]

            # --- head ---
            xyp = mpool.tile([D, 2 * BL], f32, name=f"xyp{t}", tag="sm")
            nc.tensor.matmul(xyp[:, 0:BL], FT[:], xf_prev, start=True, stop=True)
            nc.tensor.matmul(xyp[:, BL : 2 * BL], HFT[:], xf_prev,
                             start=True, stop=True)
            xp_sb = spool.tile([D, BL], f32, name=f"xp{t}", tag="xp")
            nc.vector.tensor_copy(xp_sb[:], xyp[:, 0:BL])

            nc.vector.tensor_sub(
                nnr[:, BL : 2 * BL], y_sb[:, t * BL : (t + 1) * BL],
                xyp[:, BL : 2 * BL],
            )

            sq = spool.tile([D, 2 * BL], f32, name=f"sq{t}", tag="sq")
            nc.vector.tensor_mul(sq[:], nnr[:], nnr[:])
            ss = mpool.tile([1, 2 * BL], f32, name=f"ss{t}", tag="sm")
            nc.tensor.matmul(ss[:], ones41[:], sq[:], start=True, stop=True)

            # rs = rsqrt(max(ss, 1e-24)): bit-trick seed + 2 Newton steps
            ssc = spool.tile([1, 2 * BL], f32, name=f"ssc{t}", tag="ssc")
            nc.vector.tensor_single_scalar(ssc[:], ss[:], 1e-24, op=OP.max)
            yb = spool.tile([1, 2 * BL], f32, name=f"yb{t}", tag="yb")
            nc.vector.tensor_scalar(
                yb[:].bitcast(i32), ssc[:].bitcast(i32), 1, -1,
                op0=OP.logical_shift_right, op1=OP.bitwise_xor,
            )
            nc.vector.tensor_scalar_add(
                yb[:].bitcast(i32), yb[:].bitcast(i32), MAGIC + 1
            )
            u = spool.tile([1, 2 * BL], f32, name=f"u{t}", tag="u")
            nc.vector.tensor_scalar_mul(u[:], ssc[:], -0.5)
            v = spool.tile([1, 2 * BL], f32, name=f"v{t}", tag="v")
            for _ in range(2):
                nc.vector.tensor_mul(v[:], yb[:], yb[:])
                nc.vector.tensor_mul(v[:], v[:], u[:])
                nc.vector.tensor_scalar_add(v[:], v[:], 1.5)
                nc.vector.tensor_mul(yb[:], yb[:], v[:])

            rsexp = mpool.tile([D, 2 * BL], f32, name=f"rsexp{t}", tag="sm")
            nc.tensor.matmul(rsexp[:], ones14[:], yb[:], start=True, stop=True)
            nn = spool.tile([D, 2 * BL], f32, name=f"nn{t}", tag="nn")
            nc.vector.tensor_mul(nn[:], nnr[:], rsexp[:])

            # --- a = relu(W_in @ nn) ---
            aps = mpool.tile([128, 2 * BL], f32, name=f"aps{t}", tag="sm")
            nacc = 3 if has_bias else 2
            for kc in range(2):
                o = aps[:, kc * BL : (kc + 1) * BL]
                nc.tensor.matmul(o, WinT_dx[:, kc * 128 : (kc + 1) * 128],
                                 nn[:, 0:BL], start=True, stop=False)
                nc.tensor.matmul(o, WinT_in[:, kc * 128 : (kc + 1) * 128],
                                 nn[:, BL : 2 * BL], start=False,
                                 stop=(nacc == 2))
                if has_bias:
                    nc.tensor.matmul(o, b_in_sb[:, kc : kc + 1], ones32[:],
                                     start=False, stop=True)
            a_sb = spool.tile([128, 2 * BL], f32, name=f"a{t}", tag="a")
            nc.vector.tensor_single_scalar(a_sb[:], aps[:], 0.0, op=OP.max)

            # --- gates ---
            gxh = gpool.tile([128, 6 * BL], f32, name=f"gxh{t}", tag="g")
            nbias = 1 if has_bias else 0
            for mc in range(6):
                o = gxh[:, mc * BL : (mc + 1) * BL]
                n_acc = (4 if mc < 4 else 2) + nbias
                i = 0
                for kc in range(2):
                    nc.tensor.matmul(
                        o, WhhT[:, kc * 3 * HP + mc * 128 :][:, :128],
                        h_cur[:, kc * BL : (kc + 1) * BL],
                        start=(i == 0), stop=(i == n_acc - 1),
                    )
                    i += 1
                if mc < 4:
                    for kc in range(2):
                        nc.tensor.matmul(
                            o, WihT[:, kc * 3 * HP + mc * 128 :][:, :128],
                            a_sb[:, kc * BL : (kc + 1) * BL],
                            start=False, stop=(i == n_acc - 1),
                        )
                        i += 1
                if has_bias:
                    bsl = (b_rz[:, mc : mc + 1] if mc < 4
                           else b_hh_n[:, mc - 4 : mc - 3])
                    nc.tensor.matmul(o, bsl, ones32[:], start=False, stop=True)

            xn = mpool.tile([128, 2 * BL], f32, name=f"xn{t}", tag="sm")
            for mc in range(2):
                o = xn[:, mc * BL : (mc + 1) * BL]
                for kc in range(2):
                    nc.tensor.matmul(
                        o, WihT[:, kc * 3 * HP + (4 + mc) * 128 :][:, :128],
                        a_sb[:, kc * BL : (kc + 1) * BL],
                        start=(kc == 0), stop=(kc == 1 and not has_bias),
                    )
                if has_bias:
                    nc.tensor.matmul(o, b_ih_n[:, mc : mc + 1], ones32[:],
                                     start=False, stop=True)

            rz = spool.tile([128, 4 * BL], f32, name=f"rz{t}", tag="rz")
            nc.scalar.activation(rz[:], gxh[:, 0 : 4 * BL], AF.Sigmoid)

            npre = spool.tile([128, 2 * BL], f32, name=f"npre{t}", tag="npre")
            nc.vector.tensor_mul(npre[:], rz[:, 0 : 2 * BL],
                                 gxh[:, 4 * BL : 6 * BL])
            nc.vector.tensor_add(npre[:], npre[:], xn[:])
            n_sb = spool.tile([128, 2 * BL], f32, name=f"nsb{t}", tag="nsb")
            nc.scalar.activation(n_sb[:], npre[:], AF.Tanh)

            h_new = hpool.tile([128, 2 * BL], f32, name=f"h{t}", tag="h")
            d_sb = spool.tile([128, 2 * BL], f32, name=f"dsb{t}", tag="dsb")
            nc.vector.tensor_sub(d_sb[:], h_cur[:], n_sb[:])
            nc.vector.tensor_mul(d_sb[:], rz[:, 2 * BL : 4 * BL], d_sb[:])
            nc.vector.tensor_add(h_new[:], n_sb[:], d_sb[:])
            h_cur = h_new

            # --- K and x_filt ---
            KT = mpool.tile([D, 2 * BL], f32, name=f"KT{t}", tag="sm")
            for o in range(O):
                osl = KT[:, o * BL : (o + 1) * BL]
                for kc in range(2):
                    nc.tensor.matmul(
                        osl, WoutT[:, (kc * O + o) * D : (kc * O + o + 1) * D],
                        h_new[:, kc * BL : (kc + 1) * BL],
                        start=(kc == 0), stop=(kc == 1 and not has_bias),
                    )
                if has_bias:
                    nc.tensor.matmul(
                        osl, b_out_sb[o : o + 1, :], ones32[:],
                        start=False, stop=True,
                    )

            iexp = mpool.tile([D, 2 * BL], f32, name=f"iexp{t}", tag="sm")
            nc.tensor.matmul(iexp[:, 0:BL], Eo0[:], nnr[:, BL : 2 * BL],
                             start=True, stop=True)
            nc.tensor.matmul(iexp[:, BL : 2 * BL], Eo1[:], nnr[:, BL : 2 * BL],
                             start=True, stop=True)
            iexp_sb = spool.tile([D, 2 * BL], f32, name=f"iexpsb{t}", tag="ie")
            nc.vector.tensor_copy(iexp_sb[:], iexp[:])

            t8 = spool.tile([D, 2 * BL], f32, name=f"t8{t}", tag="t8")
            nc.vector.tensor_mul(t8[:], KT[:], iexp_sb[:])
            xe = spool.tile([D, BL], f32, name=f"xe{t}", tag="xe")
            nc.vector.tensor_add(xe[:], t8[:, 0:BL], t8[:, BL : 2 * BL])
            xf_new = xs_buf[:, (t + 1) * BL : (t + 2) * BL]
            nc.vector.tensor_add(xf_new, xe[:], xp_sb[:])

            nc.sync.dma_start(
                xs_out[:, t, :].rearrange("b d -> d b"), xf_new
            )

            if t + 1 < T:
                nnr = spool.tile([D, 2 * BL], f32, name=f"nnr{t + 1}",
                                 tag="nnr", bufs=3)
                nc.vector.tensor_sub(nnr[:, 0:BL], xf_new, xp_sb[:])

    nc.compile()
    return nc


# --------------------------------------------------------------------------
# Host entry
# --------------------------------------------------------------------------
def _make_in_maps(inputs):
    w = _prep_weights(
        inputs["F_mat"], inputs["H_mat"], inputs["W_in"], inputs["b_in"],
        inputs["W_ih"], inputs["W_hh"], inputs["b_ih"], inputs["b_hh"],
        inputs["W_out"], inputs["b_out"],
    )
    y_seq = np.asarray(inputs["y_seq"], np.float32)
    common = {
        "WhhT": w["WhhT"], "WihT": w["WihT"], "WinT_dx": w["WinT_dx"],
        "WinT_in": w["WinT_in"], "WoutT": w["WoutT"], "FT": w["FT"],
        "HFT": w["HFT"], "ones41": w["ones41"], "ones14": w["ones14"],
        "Eo0": w["Eo0"], "Eo1": w["Eo1"],
    }
    if w["has_bias"]:
        common.update(w["biases"])
    in_maps = []
    for c in range(NCORES):
        ys = y_seq[c * BL : (c + 1) * BL]              # [BL, T, O]
        yT = np.zeros((D, T * BL), np.float32)
        yT[0:O] = ys.transpose(2, 1, 0).reshape(O, T * BL)
        m = dict(common)
        m["y_in"] = yT
        in_maps.append(m)
    return in_maps, w["has_bias"]


def _get_nc(has_bias):
    key = ("nc", has_bias)
    if key not in _CACHE:
        _CACHE[key] = build_nc(has_bias)
    return _CACHE[key]


def _get_exec(has_bias):
    """Build (once) a persistently-jitted 8-core executor for the program."""
    key = ("exec", has_bias)
    if key in _CACHE:
        return _CACHE[key]

    import jax
    from jax.sharding import Mesh, PartitionSpec
    from jax.experimental.shard_map import shard_map
    from concourse import mybir as _mybir
    from concourse.bass2jax import (
        _bass_exec_p, install_neuronx_cc_hook, partition_id_tensor,
    )

    nc = _get_nc(has_bias)
    install_neuronx_cc_hook()

    pid_name = nc.partition_id_tensor.name if nc.partition_id_tensor else None
    in_names, out_names, out_avals, zero_outs = [], [], [], []
    for alloc in nc.m.functions[0].allocations:
        if not isinstance(alloc, _mybir.MemoryLocationSet):
            continue
        name = alloc.memorylocations[0].name
        if alloc.kind == "ExternalInput":
            if name != pid_name:
                in_names.append(name)
        elif alloc.kind == "ExternalOutput":
            shape = tuple(alloc.tensor_shape)
            dtype = _mybir.dt.np(alloc.dtype)
            out_names.append(name)
            out_avals.append(jax.core.ShapedArray(shape, dtype))
            zero_outs.append(np.zeros(shape, dtype))
    n_params = len(in_names)
    all_in_names = in_names + out_names
    if pid_name is not None:
        all_in_names = all_in_names + [pid_name]

    def _body(*args):
        operands = list(args)
        if pid_name is not None:
            operands.append(partition_id_tensor())
        outs = _bass_exec_p.bind(
            *operands,
            out_avals=tuple(out_avals),
            in_names=tuple(all_in_names),
            out_names=tuple(out_names),
            lowering_input_output_aliases=(),
            sim_require_finite=True,
            sim_require_nnan=True,
            nc=nc,
        )
        return tuple(outs)

    devices = jax.devices()[:NCORES]
    mesh = Mesh(np.asarray(devices), ("core",))
    nin = n_params + len(out_names)
    sharded = jax.jit(
        shard_map(
            _body, mesh=mesh,
            in_specs=(PartitionSpec("core"),) * nin,
            out_specs=(PartitionSpec("core"),) * len(out_names),
            check_rep=False,
        ),
        keep_unused=True,
    )
    state = dict(
        fn=sharded, in_names=in_names, out_names=out_names,
        out_avals=out_avals, zero_outs=zero_outs, n_params=n_params,
    )
    _CACHE[key] = state
    return state


def _run(in_maps, has_bias):
    st = _get_exec(has_bias)
    concat_in = [
        np.concatenate([np.asarray(m[name]) for m in in_maps], axis=0)
        for name in st["in_names"]
    ]
    concat_zero = [
        np.zeros((NCORES * z.shape[0], *z.shape[1:]), z.dtype)
        for z in st["zero_outs"]
    ]
    out = st["fn"](*concat_in, *concat_zero)
    res = {}
    for i, name in enumerate(st["out_names"]):
        arr = np.asarray(out[i])
        res[name] = arr.reshape(NCORES, *st["out_avals"][i].shape)
    return res


def kernel(**inputs):
    in_maps, has_bias = _make_in_maps(inputs)
    res = _run(in_maps, has_bias)
    xs = np.concatenate(list(res["xs_out"]), axis=0).astype(np.float32)
    Ps = np.zeros((B, T, D, D), np.float32)
    return xs, Ps
